# revision 6
# baseline (speedup 1.0000x reference)
"""Trainium2 Bass kernel for nn_AutoRNN: vanilla RNN over T=16384 steps.

  h_t = tanh(x_t @ Wx + h_{t-1} @ Wh + bh);  y_t = h_t @ Wy + by
  returns (hidden [T,1024], outputs [T,1024])

Strategy (8 NeuronCores, SPMD, no collectives):
  Phase 1 (redundant on every core): A = X @ Wx + bh  -> scratch DRAM (f32r matmuls)
  Phase 2 (redundant on every core): sequential chain h_t = tanh(A_t + Wh^T-free
           matvec) fully on-device, fp32, tensor-engine col-strip tiling;
           H streamed to scratch DRAM transposed (HT).
  Phase 3 (sharded by core): Y = H @ Wy + by and H output, each core does its
           T/8 rows using partition_id-offset DMA reads of HT.
"""
import sys

sys.path.insert(0, "/opt/trn_rl_repo")

from contextlib import ExitStack

import numpy as np

import concourse.bass as bass
import concourse.tile as tile
from concourse import bacc, mybir
from concourse.bass import ds
from concourse.bass_utils import run_bass_kernel_spmd

FP32 = mybir.dt.float32
F32R = mybir.dt.float32r
AF = mybir.ActivationFunctionType

T, D = 16384, 1024
NCORES = 8
SHARD = T // NCORES  # 2048
B = 64  # chain block (steps per For_i iteration)
NB = T // B
TILED_CHAIN = True  # 4-col-strip M=32 tiling vs straight M=128


def build_program(t_total=T, blk=B):
    """Builds the Bass program. Returns nc."""
    global T, B, NB, SHARD
    T, B = t_total, blk
    NB = T // B
    SHARD = T // NCORES
    nc = bacc.Bacc("TRN2", target_bir_lowering=False, debug=False, num_devices=NCORES)

    x_d = nc.dram_tensor("x", (T, D), FP32, kind="ExternalInput").ap()
    wx_d = nc.dram_tensor("wx", (D, D), FP32, kind="ExternalInput").ap()
    wh_d = nc.dram_tensor("wh", (D, D), FP32, kind="ExternalInput").ap()
    wy_d = nc.dram_tensor("wy", (D, D), FP32, kind="ExternalInput").ap()
    bh_d = nc.dram_tensor("bh", (1, D), FP32, kind="ExternalInput").ap()
    by_d = nc.dram_tensor("by", (1, D), FP32, kind="ExternalInput").ap()
    i8_d = nc.dram_tensor("i8", (8, 8), FP32, kind="ExternalInput").ap()
    i128_d = nc.dram_tensor("i128", (128, 128), FP32, kind="ExternalInput").ap()
    ones_d = nc.dram_tensor("ones", (1, 128), FP32, kind="ExternalInput").ap()

    hout_d = nc.dram_tensor("hout", (SHARD, D), FP32, kind="ExternalOutput").ap()
    yout_d = nc.dram_tensor("yout", (SHARD, D), FP32, kind="ExternalOutput").ap()

    a_sc = nc.dram_tensor("a_sc", (T, D), FP32, kind="Internal").ap()
    ht_sc = nc.dram_tensor("ht_sc", (D, T), FP32, kind="Internal").ap()

    with tile.TileContext(nc) as tc:
        build_kernel(
            tc, x_d, wx_d, wh_d, wy_d, bh_d, by_d, i8_d, i128_d, ones_d,
            hout_d, yout_d, a_sc, ht_sc,
        )
    nc.compile()
    return nc


def build_kernel(tc, x_d, wx_d, wh_d, wy_d, bh_d, by_d, i8_d, i128_d, ones_d,
                 hout_d, yout_d, a_sc, ht_sc):
    nc = tc.nc

    # ---------------- Phase 1: A = X @ Wx + bh (f32r) ----------------
    with ExitStack() as p1:
        wpool = p1.enter_context(tc.tile_pool(name="p1w", bufs=1))
        xpool = p1.enter_context(tc.tile_pool(name="p1x", bufs=3))
        xtpool = p1.enter_context(tc.tile_pool(name="p1xt", bufs=3))
        apool = p1.enter_context(tc.tile_pool(name="p1a", bufs=3))
        pst = p1.enter_context(tc.tile_pool(name="p1pst", bufs=2, space="PSUM"))
        psa = p1.enter_context(tc.tile_pool(name="p1psa", bufs=4, space="PSUM"))

        # Wx chunk layout: Wx[k, j] -> partition k%128, free (k//128)*1024 + j
        wx_sb = wpool.tile([128, 8 * D], F32R)
        nc.sync.dma_start(
            wx_sb[:].rearrange("p (i j) -> p i j", i=8),
            wx_d.rearrange("(i p) j -> p i j", p=128).bitcast(F32R),
        )
        bh_sb = wpool.tile([1, D], F32R)
        nc.sync.dma_start(bh_sb[:], bh_d[:].bitcast(F32R))
        ones_sb = wpool.tile([1, 128], F32R)
        nc.sync.dma_start(ones_sb[:], ones_d[:].bitcast(F32R))
        i128_sb = wpool.tile([128, 128], FP32)
        nc.sync.dma_start(i128_sb[:], i128_d[:])

        for tt in range(T // 128):
            xt_tile = xpool.tile([128, D], FP32, tag="x")
            nc.sync.dma_start(xt_tile[:], x_d[ds(tt * 128, 128), :])
            # transpose X tile chunks: xt[p, 128*i + t] = X[tt*128+t, 128*i+p]
            xtr = xtpool.tile([128, D], F32R, tag="xt")
            for half in range(2):
                pt = pst.tile([128, 512], FP32, tag="pst")
                for c in range(4):
                    i = half * 4 + c
                    nc.tensor.transpose(
                        pt[:, 128 * c : 128 * c + 128],
                        xt_tile[:, 128 * i : 128 * i + 128],
                        i128_sb[:],
                    )
                nc.vector.tensor_copy(
                    xtr[:, 512 * half : 512 * half + 512], pt[:]
                )
            # A tile = XT.T @ Wx + bh
            a_tile = apool.tile([128, D], FP32, tag="a")
            for jh in range(2):
                pa = psa.tile([128, 512], FP32, tag="psa")
                for i in range(8):
                    nc.tensor.matmul(
                        pa[:],
                        xtr[:, 128 * i : 128 * i + 128],
                        wx_sb[:, 1024 * i + 512 * jh : 1024 * i + 512 * jh + 512],
                        start=(i == 0),
                        stop=False,
                    )
                nc.tensor.matmul(
                    pa[:],
                    ones_sb[:],
                    bh_sb[:, 512 * jh : 512 * jh + 512],
                    start=False,
                    stop=True,
                )
                nc.vector.tensor_copy(a_tile[:, 512 * jh : 512 * jh + 512], pa[:])
            nc.sync.dma_start(a_sc[ds(tt * 128, 128), :], a_tile[:])

    tc.strict_bb_all_engine_barrier()

    # ---------------- Phase 2: the chain ----------------
    with ExitStack() as p2:
        wpool = p2.enter_context(tc.tile_pool(name="chw", bufs=1))
        abuf = p2.enter_context(tc.tile_pool(name="chab", bufs=2))
        hpool = p2.enter_context(tc.tile_pool(name="chh", bufs=1))
        psc = p2.enter_context(tc.tile_pool(name="chps", bufs=2, space="PSUM"))

        # Wh chunk layout [128, 8192]
        wh_sb = wpool.tile([128, 8 * D], FP32)
        nc.sync.dma_start(
            wh_sb[:].rearrange("p (i j) -> p i j", i=8),
            wh_d.rearrange("(i p) j -> p i j", p=128),
        )
        i8_sb = wpool.tile([8, 8], FP32)
        nc.sync.dma_start(i8_sb[:], i8_d[:])

        # hstage: [128, 8 * (B+1)]; slot layout per chunk i: cols i*(B+1)+s,
        # s=0 is carry-in h_{t0-1}, s=dt+1 holds h_{t0+dt}.
        S = B + 1
        hstage = hpool.tile([128, 8 * S], FP32)
        hview = hstage[:].rearrange("p (i s) -> p i s", s=S)
        nc.vector.memset(hview[:, :, 0], 0.0)

        with tc.For_i(0, T, B, hint_engines=(mybir.EngineType.PE,)) as t0:
            ablk = abuf.tile([8, B * 128], FP32, tag="ablk")
            nc.sync.dma_start(
                ablk[:].rearrange("p (t m) -> p t m", m=128),
                a_sc[ds(t0, B), :].rearrange("t (i m) -> i t m", i=8),
            )
            for dt in range(B):
                ps = psc.tile([128, 512], FP32, tag="chps")
                # open accumulation group with the A-fold (covers all 128 parts)
                nc.tensor.matmul(
                    ps[:, 0:8],
                    ablk[:, 128 * dt : 128 * dt + 128],
                    i8_sb[:],
                    start=True,
                    stop=False,
                    skip_group_check=True,
                )
                if TILED_CHAIN:
                    for jm in range(32):
                        c = jm % 4
                        col = jm // 4
                        for i in range(8):
                            nc.tensor.matmul(
                                ps[32 * c : 32 * c + 32, col : col + 1],
                                wh_sb[:, 1024 * i + 32 * jm : 1024 * i + 32 * jm + 32],
                                hstage[:, i * S + dt : i * S + dt + 1],
                                start=False,
                                stop=(jm == 31 and i == 7),
                                tile_position=(0, 32 * c),
                                skip_group_check=True,
                            )
                else:
                    for jc in range(8):
                        for i in range(8):
                            nc.tensor.matmul(
                                ps[:, jc : jc + 1],
                                wh_sb[:, 1024 * i + 128 * jc : 1024 * i + 128 * jc + 128],
                                hstage[:, i * S + dt : i * S + dt + 1],
                                start=False,
                                stop=(jc == 7 and i == 7),
                                skip_group_check=True,
                            )
                nc.scalar.activation(hview[:, :, dt + 1], ps[:, 0:8], AF.Tanh)
            # stream H block out (transposed layout), carry h into slot 0
            nc.sync.dma_start(
                ht_sc.rearrange("(i p) t -> p i t", p=128)[:, :, ds(t0, B)],
                hview[:, :, 1:],
            )
            nc.vector.tensor_copy(hview[:, :, 0], hview[:, :, B])

    tc.strict_bb_all_engine_barrier()

    # ---------------- Phase 3: Y = H @ Wy + by, H output (sharded) ----------
    with ExitStack() as p3:
        wpool = p3.enter_context(tc.tile_pool(name="p3w", bufs=1))
        htpool = p3.enter_context(tc.tile_pool(name="p3ht", bufs=16))
        opool = p3.enter_context(tc.tile_pool(name="p3o", bufs=4))
        psy = p3.enter_context(tc.tile_pool(name="p3psy", bufs=4, space="PSUM"))
        psh = p3.enter_context(tc.tile_pool(name="p3psh", bufs=2, space="PSUM"))

        wy_sb = wpool.tile([128, 8 * D], F32R)
        nc.sync.dma_start(
            wy_sb[:].rearrange("p (i j) -> p i j", i=8),
            wy_d.rearrange("(i p) j -> p i j", p=128).bitcast(F32R),
        )
        by_sb = wpool.tile([1, D], F32R)
        nc.sync.dma_start(by_sb[:], by_d[:].bitcast(F32R))
        ones_sb = wpool.tile([1, 128], F32R)
        nc.sync.dma_start(ones_sb[:], ones_d[:].bitcast(F32R))
        i128r_sb = wpool.tile([128, 128], F32R)
        nc.sync.dma_start(i128r_sb[:], i128_d[:].bitcast(F32R))

        pid = nc.partition_id()
        base = pid * SHARD

        for tt in range(SHARD // 128):
            hts = []
            for i in range(8):
                htt = htpool.tile([128, 128], F32R, tag="ht")
                nc.sync.dma_start(
                    htt[:],
                    ht_sc[ds(128 * i, 128), ds(base + 128 * tt, 128)].bitcast(F32R),
                )
                hts.append(htt)
            # Y tile
            y_tile = opool.tile([128, D], FP32, tag="y")
            for jh in range(2):
                pa = psy.tile([128, 512], FP32, tag="psy")
                for i in range(8):
                    nc.tensor.matmul(
                        pa[:],
                        hts[i][:],
                        wy_sb[:, 1024 * i + 512 * jh : 1024 * i + 512 * jh + 512],
                        start=(i == 0),
                        stop=False,
                    )
                nc.tensor.matmul(
                    pa[:],
                    ones_sb[:],
                    by_sb[:, 512 * jh : 512 * jh + 512],
                    start=False,
                    stop=True,
                )
                nc.vector.tensor_copy(y_tile[:, 512 * jh : 512 * jh + 512], pa[:])
            nc.sync.dma_start(yout_d[ds(128 * tt, 128), :], y_tile[:])
            # H tile via PE transpose of the already-loaded HT tiles
            h_tile = opool.tile([128, D], FP32, tag="h")
            for half in range(2):
                pt = psh.tile([128, 512], F32R, tag="psh")
                for c in range(4):
                    i = half * 4 + c
                    nc.tensor.transpose(
                        pt[:, 128 * c : 128 * c + 128], hts[i][:], i128r_sb[:]
                    )
                nc.vector.tensor_copy(
                    h_tile[:, 512 * half : 512 * half + 512],
                    pt[:].bitcast(FP32),
                )
            nc.sync.dma_start(hout_d[ds(128 * tt, 128), :], h_tile[:])


_NC_CACHE = {}


def kernel(**inputs):
    X = np.ascontiguousarray(np.asarray(inputs["X_seq"], dtype=np.float32))
    Wx = np.ascontiguousarray(np.asarray(inputs["Wx"], dtype=np.float32))
    Wh = np.ascontiguousarray(np.asarray(inputs["Wh"], dtype=np.float32))
    Wy = np.ascontiguousarray(np.asarray(inputs["Wy"], dtype=np.float32))
    bh = np.asarray(inputs["bh"], dtype=np.float32).reshape(1, D)
    by = np.asarray(inputs["by"], dtype=np.float32).reshape(1, D)

    if "nc" not in _NC_CACHE:
        _NC_CACHE["nc"] = build_program()
    nc = _NC_CACHE["nc"]

    feed = {
        "x": X, "wx": Wx, "wh": Wh, "wy": Wy, "bh": bh, "by": by,
        "i8": np.eye(8, dtype=np.float32),
        "i128": np.eye(128, dtype=np.float32),
        "ones": np.ones((1, 128), dtype=np.float32),
    }
    in_maps = [dict(feed) for _ in range(NCORES)]
    res = run_bass_kernel_spmd(nc, in_maps, list(range(NCORES)))
    H = np.concatenate([res.results[c]["hout"] for c in range(NCORES)], axis=0)
    Y = np.concatenate([res.results[c]["yout"] for c in range(NCORES)], axis=0)
    return (H, Y)


if __name__ == "__main__":
    rng = np.random.default_rng(0)
    ins = {
        "X_seq": rng.standard_normal((T, D), dtype=np.float32),
        "Wx": (rng.standard_normal((D, D)) * 0.03).astype(np.float32),
        "Wh": (rng.standard_normal((D, D)) * 0.03).astype(np.float32),
        "Wy": (rng.standard_normal((D, D)) * 0.03).astype(np.float32),
        "bh": (rng.standard_normal(D) * 0.03).astype(np.float32),
        "by": (rng.standard_normal(D) * 0.03).astype(np.float32),
    }
    H, Y = kernel(**ins)
    print("H", H.shape, "Y", Y.shape)


# revision 8
# speedup vs baseline: 2.1522x; 2.1522x over previous
"""Trainium2 Bass kernel for nn_AutoRNN: vanilla RNN over T=16384 steps.

  h_t = tanh(x_t @ Wx + h_{t-1} @ Wh + bh);  y_t = h_t @ Wy + by
  returns (hidden [T,1024], outputs [T,1024])

Strategy (8 NeuronCores, SPMD, no collectives):
  Phase 1 (redundant on every core): A = X @ Wx + bh  -> scratch DRAM (f32r matmuls)
  Phase 2 (redundant on every core): sequential chain h_t = tanh(A_t + Wh^T-free
           matvec) fully on-device, fp32, tensor-engine col-strip tiling;
           H streamed to scratch DRAM transposed (HT).
  Phase 3 (sharded by core): Y = H @ Wy + by and H output, each core does its
           T/8 rows using partition_id-offset DMA reads of HT.
"""
import sys

sys.path.insert(0, "/opt/trn_rl_repo")

from contextlib import ExitStack

import numpy as np

import concourse.bass as bass
import concourse.tile as tile
from concourse import bacc, mybir
from concourse.bass import ds
from concourse.bass_utils import run_bass_kernel_spmd

FP32 = mybir.dt.float32
F32R = mybir.dt.float32r
AF = mybir.ActivationFunctionType

T, D = 16384, 1024
NCORES = 8
SHARD = T // NCORES  # 2048
B = 64  # chain block (steps per For_i iteration)
NB = T // B
TILED_CHAIN = True  # 4-col-strip M=32 tiling vs straight M=128


def build_program(t_total=T, blk=B):
    """Builds the Bass program. Returns nc."""
    global T, B, NB, SHARD
    T, B = t_total, blk
    NB = T // B
    SHARD = T // NCORES
    nc = bacc.Bacc("TRN2", target_bir_lowering=False, debug=False, num_devices=NCORES)

    x_d = nc.dram_tensor("x", (T, D), FP32, kind="ExternalInput").ap()
    wx_d = nc.dram_tensor("wx", (D, D), FP32, kind="ExternalInput").ap()
    wh_d = nc.dram_tensor("wh", (D, D), FP32, kind="ExternalInput").ap()
    wy_d = nc.dram_tensor("wy", (D, D), FP32, kind="ExternalInput").ap()
    bh_d = nc.dram_tensor("bh", (1, D), FP32, kind="ExternalInput").ap()
    by_d = nc.dram_tensor("by", (1, D), FP32, kind="ExternalInput").ap()
    i8_d = nc.dram_tensor("i8", (8, 8), FP32, kind="ExternalInput").ap()
    i128_d = nc.dram_tensor("i128", (128, 128), FP32, kind="ExternalInput").ap()
    ones_d = nc.dram_tensor("ones", (128, 128), FP32, kind="ExternalInput").ap()

    hout_d = nc.dram_tensor("hout", (SHARD, D), FP32, kind="ExternalOutput").ap()
    yout_d = nc.dram_tensor("yout", (SHARD, D), FP32, kind="ExternalOutput").ap()

    a_sc = nc.dram_tensor("a_sc", (T, D), FP32, kind="Internal").ap()
    ht_sc = nc.dram_tensor("ht_sc", (D, T), FP32, kind="Internal").ap()

    with tile.TileContext(nc) as tc:
        build_kernel(
            tc, x_d, wx_d, wh_d, wy_d, bh_d, by_d, i8_d, i128_d, ones_d,
            hout_d, yout_d, a_sc, ht_sc,
        )
    nc.compile()
    return nc


def build_kernel(tc, x_d, wx_d, wh_d, wy_d, bh_d, by_d, i8_d, i128_d, ones_d,
                 hout_d, yout_d, a_sc, ht_sc):
    nc = tc.nc

    # ---------------- Phase 1: A = X @ Wx + bh (f32r) ----------------
    with ExitStack() as p1:
        wpool = p1.enter_context(tc.tile_pool(name="p1w", bufs=1))
        xpool = p1.enter_context(tc.tile_pool(name="p1x", bufs=3))
        xtpool = p1.enter_context(tc.tile_pool(name="p1xt", bufs=3))
        apool = p1.enter_context(tc.tile_pool(name="p1a", bufs=3))
        pst = p1.enter_context(tc.tile_pool(name="p1pst", bufs=2, space="PSUM"))
        psa = p1.enter_context(tc.tile_pool(name="p1psa", bufs=4, space="PSUM"))

        # Wx chunk layout: Wx[k, j] -> partition k%128, free (k//128)*1024 + j
        wx_sb = wpool.tile([128, 8 * D], F32R)
        nc.sync.dma_start(
            wx_sb[:].rearrange("p (i j) -> p i j", i=8),
            wx_d.rearrange("(i p) j -> p i j", p=128).bitcast(F32R),
        )
        bh_sb = wpool.tile([1, D], F32R)
        nc.sync.dma_start(bh_sb[:], bh_d[:].bitcast(F32R))
        ones_sb = wpool.tile([1, 128], F32R)
        nc.sync.dma_start(ones_sb[:], ones_d[0:1, :].bitcast(F32R))
        i128_sb = wpool.tile([128, 128], FP32)
        nc.sync.dma_start(i128_sb[:], i128_d[:])

        for tt in range(T // 128):
            xt_tile = xpool.tile([128, D], FP32, tag="x")
            nc.sync.dma_start(xt_tile[:], x_d[ds(tt * 128, 128), :])
            # transpose X tile chunks: xt[p, 128*i + t] = X[tt*128+t, 128*i+p]
            xtr = xtpool.tile([128, D], F32R, tag="xt")
            for half in range(2):
                pt = pst.tile([128, 512], FP32, tag="pst")
                for c in range(4):
                    i = half * 4 + c
                    nc.tensor.transpose(
                        pt[:, 128 * c : 128 * c + 128],
                        xt_tile[:, 128 * i : 128 * i + 128],
                        i128_sb[:],
                    )
                nc.vector.tensor_copy(
                    xtr[:, 512 * half : 512 * half + 512], pt[:]
                )
            # A tile = XT.T @ Wx + bh
            a_tile = apool.tile([128, D], FP32, tag="a")
            for jh in range(2):
                pa = psa.tile([128, 512], FP32, tag="psa")
                for i in range(8):
                    nc.tensor.matmul(
                        pa[:],
                        xtr[:, 128 * i : 128 * i + 128],
                        wx_sb[:, 1024 * i + 512 * jh : 1024 * i + 512 * jh + 512],
                        start=(i == 0),
                        stop=False,
                    )
                nc.tensor.matmul(
                    pa[:],
                    ones_sb[:],
                    bh_sb[:, 512 * jh : 512 * jh + 512],
                    start=False,
                    stop=True,
                )
                nc.vector.tensor_copy(a_tile[:, 512 * jh : 512 * jh + 512], pa[:])
            nc.sync.dma_start(a_sc[ds(tt * 128, 128), :], a_tile[:])

    tc.strict_bb_all_engine_barrier()

    # ---------------- Phase 2: the chain (h-stationary, fp32, col strips) ----
    with ExitStack() as p2:
        wpool = p2.enter_context(tc.tile_pool(name="chw", bufs=1))
        abuf = p2.enter_context(tc.tile_pool(name="chab", bufs=1))
        hpool = p2.enter_context(tc.tile_pool(name="chh", bufs=1))
        hbpool = p2.enter_context(tc.tile_pool(name="chhb", bufs=2))
        psc = p2.enter_context(tc.tile_pool(name="chps", bufs=2, space="PSUM"))
        ptp = p2.enter_context(tc.tile_pool(name="chpt", bufs=2, space="PSUM"))

        # Wh chunk layout [128, 8192]: (p, i*1024 + j) = Wh[i*128+p, j]
        wh_sb = wpool.tile([128, 8 * D], FP32)
        nc.sync.dma_start(
            wh_sb[:].rearrange("p (i j) -> p i j", i=8),
            wh_d.rearrange("(i p) j -> p i j", p=128),
        )
        ones32 = wpool.tile([128, 32], FP32)
        nc.sync.dma_start(ones32[:], ones_d[:, 0:32])
        i128c = wpool.tile([128, 128], FP32)
        nc.sync.dma_start(i128c[:], i128_d[:])

        # ping-pong replicated h (slot = dt%2); chunk i at col 128*(i%2)+32*(i//2)
        hrep = hpool.tile([128, 512], FP32)
        nc.vector.memset(hrep[:, 0:256], 0.0)
        # H staging for block DMA out: (p, i*B + t)
        hstage = hpool.tile([128, 8 * B], FP32)

        def chunk_col(i):
            return 128 * (i % 2) + 32 * (i // 2)

        with tc.For_i(0, T, B, hint_engines=(mybir.EngineType.PE,)) as t0:
            ablk = abuf.tile([128, B * 256], FP32, tag="ablk")
            nc.sync.dma_start(
                ablk[:].rearrange("(g q) (t n) -> g q t n", q=32, n=256)[:, 0, :, :],
                a_sc[ds(t0, B), :].rearrange("t (g n) -> g t n", g=4),
            )
            for dt in range(B):
                cur = 256 * (dt % 2)
                nxt = 256 * ((dt + 1) % 2)
                ps = psc.tile([128, 512], FP32, tag="chps")
                for g in range(4):
                    nc.tensor.matmul(
                        ps[32 * g : 32 * g + 32, 0:256],
                        ones32[32 * g : 32 * g + 1, :],
                        ablk[32 * g : 32 * g + 1, 256 * dt : 256 * dt + 256],
                        start=True, stop=False,
                        tile_position=(32 * g, 32 * g),
                        skip_group_check=True,
                    )
                for i in range(8):
                    for g in range(4):
                        nc.tensor.matmul(
                            ps[32 * g : 32 * g + 32, 0:256],
                            hrep[:, cur + chunk_col(i) : cur + chunk_col(i) + 32],
                            wh_sb[:, 1024 * i + 256 * g : 1024 * i + 256 * g + 256],
                            start=False, stop=(i == 7),
                            tile_position=(0, 32 * g),
                            skip_group_check=True,
                        )
                hb = hbpool.tile([128, 256], FP32, tag="hb")
                nc.scalar.activation(hb[:], ps[:, 0:256], AF.Tanh)
                pt = ptp.tile([128, 512], FP32, tag="chpt")
                for half in range(2):
                    nc.tensor.transpose(
                        pt[:, 128 * half : 128 * half + 128],
                        hb[:, 128 * half : 128 * half + 128],
                        i128c[:],
                    )
                nc.vector.tensor_copy(hrep[:, nxt : nxt + 256], pt[:, 0:256])
                # gather unique chunk columns into H staging (off critical path)
                nc.vector.tensor_copy(
                    hstage[:].rearrange("p (c h t) -> p c h t", c=4, h=2)[:, :, :, dt],
                    pt[:, 0:256].rearrange("p (h c r) -> p c h r", h=2, c=4)[:, :, :, 0],
                )
            nc.sync.dma_start(
                ht_sc.rearrange("(i p) t -> p i t", p=128)[:, :, ds(t0, B)],
                hstage[:].rearrange("p (i t) -> p i t", t=B),
            )

    # ---------------- Phase 3: Y = H @ Wy + by, H output (sharded) ----------
    with ExitStack() as p3:
        wpool = p3.enter_context(tc.tile_pool(name="p3w", bufs=1))
        htpool = p3.enter_context(tc.tile_pool(name="p3ht", bufs=16))
        opool = p3.enter_context(tc.tile_pool(name="p3o", bufs=4))
        psy = p3.enter_context(tc.tile_pool(name="p3psy", bufs=4, space="PSUM"))
        psh = p3.enter_context(tc.tile_pool(name="p3psh", bufs=2, space="PSUM"))

        wy_sb = wpool.tile([128, 8 * D], F32R)
        nc.sync.dma_start(
            wy_sb[:].rearrange("p (i j) -> p i j", i=8),
            wy_d.rearrange("(i p) j -> p i j", p=128).bitcast(F32R),
        )
        by_sb = wpool.tile([1, D], F32R)
        nc.sync.dma_start(by_sb[:], by_d[:].bitcast(F32R))
        ones_sb = wpool.tile([1, 128], F32R)
        nc.sync.dma_start(ones_sb[:], ones_d[0:1, :].bitcast(F32R))
        i128r_sb = wpool.tile([128, 128], F32R)
        nc.sync.dma_start(i128r_sb[:], i128_d[:].bitcast(F32R))

        pid = nc.partition_id()
        base = pid * SHARD

        for tt in range(SHARD // 128):
            hts = []
            for i in range(8):
                htt = htpool.tile([128, 128], F32R, tag="ht")
                nc.sync.dma_start(
                    htt[:],
                    ht_sc[ds(128 * i, 128), ds(base + 128 * tt, 128)].bitcast(F32R),
                )
                hts.append(htt)
            # Y tile
            y_tile = opool.tile([128, D], FP32, tag="y")
            for jh in range(2):
                pa = psy.tile([128, 512], FP32, tag="psy")
                for i in range(8):
                    nc.tensor.matmul(
                        pa[:],
                        hts[i][:],
                        wy_sb[:, 1024 * i + 512 * jh : 1024 * i + 512 * jh + 512],
                        start=(i == 0),
                        stop=False,
                    )
                nc.tensor.matmul(
                    pa[:],
                    ones_sb[:],
                    by_sb[:, 512 * jh : 512 * jh + 512],
                    start=False,
                    stop=True,
                )
                nc.vector.tensor_copy(y_tile[:, 512 * jh : 512 * jh + 512], pa[:])
            nc.sync.dma_start(yout_d[ds(128 * tt, 128), :], y_tile[:])
            # H tile via PE transpose of the already-loaded HT tiles
            h_tile = opool.tile([128, D], FP32, tag="h")
            for half in range(2):
                pt = psh.tile([128, 512], F32R, tag="psh")
                for c in range(4):
                    i = half * 4 + c
                    nc.tensor.transpose(
                        pt[:, 128 * c : 128 * c + 128], hts[i][:], i128r_sb[:]
                    )
                nc.vector.tensor_copy(
                    h_tile[:, 512 * half : 512 * half + 512],
                    pt[:].bitcast(FP32),
                )
            nc.sync.dma_start(hout_d[ds(128 * tt, 128), :], h_tile[:])


_NC_CACHE = {}


def kernel(**inputs):
    X = np.ascontiguousarray(np.asarray(inputs["X_seq"], dtype=np.float32))
    Wx = np.ascontiguousarray(np.asarray(inputs["Wx"], dtype=np.float32))
    Wh = np.ascontiguousarray(np.asarray(inputs["Wh"], dtype=np.float32))
    Wy = np.ascontiguousarray(np.asarray(inputs["Wy"], dtype=np.float32))
    bh = np.asarray(inputs["bh"], dtype=np.float32).reshape(1, D)
    by = np.asarray(inputs["by"], dtype=np.float32).reshape(1, D)

    if "nc" not in _NC_CACHE:
        _NC_CACHE["nc"] = build_program()
    nc = _NC_CACHE["nc"]

    feed = {
        "x": X, "wx": Wx, "wh": Wh, "wy": Wy, "bh": bh, "by": by,
        "i8": np.eye(8, dtype=np.float32),
        "i128": np.eye(128, dtype=np.float32),
        "ones": np.ones((128, 128), dtype=np.float32),
    }
    in_maps = [dict(feed) for _ in range(NCORES)]
    res = run_bass_kernel_spmd(nc, in_maps, list(range(NCORES)))
    H = np.concatenate([res.results[c]["hout"] for c in range(NCORES)], axis=0)
    Y = np.concatenate([res.results[c]["yout"] for c in range(NCORES)], axis=0)
    return (H, Y)


if __name__ == "__main__":
    rng = np.random.default_rng(0)
    ins = {
        "X_seq": rng.standard_normal((T, D), dtype=np.float32),
        "Wx": (rng.standard_normal((D, D)) * 0.03).astype(np.float32),
        "Wh": (rng.standard_normal((D, D)) * 0.03).astype(np.float32),
        "Wy": (rng.standard_normal((D, D)) * 0.03).astype(np.float32),
        "bh": (rng.standard_normal(D) * 0.03).astype(np.float32),
        "by": (rng.standard_normal(D) * 0.03).astype(np.float32),
    }
    H, Y = kernel(**ins)
    print("H", H.shape, "Y", Y.shape)
